# revision 9
# baseline (speedup 1.0000x reference)
"""SpGAT_Conv Trainium2 kernel: 8-core SPMD spectral GNN conv.

Math (reference):
    a = softmax(alpha)
    pre = x @ W                                   [N, D]
    out_low  = s0 @ (a0 * (s1 @ pre))             [N, D]
    out_high = s2 @ (a1 * (s3 @ pre))             [N, D]
    out = relu(max(out_low, out_high) + bias)

Re-association: t = S @ (x @ W) == (S @ x) @ W with S = concat(s1, s3).
Row-sharding t's rows across 8 cores makes the x@W work perfectly sharded
too (it rides on each core's own 1024 rows of u = S_c @ x), cutting
per-core PE work to the distribution-optimal 1056 big-matmul equivalents.

    step 1: u_c^T = x^T S_c^T accumulated over n-chunks; stationary = x
            chunks (natural layout), moving = S_c^T strips.  One 512-wide
            sweep (A, i-cols 0..511) then two 256-wide sweeps (B1, B2) —
            narrower late sweeps stage t earlier so the AllGather chain
            (whose CC engine only comes alive ~60-85us in) finishes well
            before phase 3 needs its data.
    step 2: t rows = u^T slices stationary, W moving; each sweep's rows
            are staged + AllGathered immediately (AG0/AG1 after sweep A,
            AG2 after B1, AG3 after B2).
    phase 3: out_c = relu(max(s0_c@t1 + bias, s2_c@t3 + bias)), high band
             then low band in AG-arrival order g 0,1,2,3.  softmax(alpha)
             is folded into s0/s2 on the host, bias is passed
             pre-broadcast [128, D] and seeded into PSUM, so the epilogue
             per strip is one ACT relu + one DVE max.  The last two
             low-band strips run nt-major so the 8 accumulators complete
             staggered and the epilogue pipelines behind the matmuls.

Schedule notes (v3):
  - PE warm-up junk matmuls keep the HAM activity window busy from
    ~6.5us so the first real matmuls aren't clock-gated.
  - Steady-state matmul rate is ~263ns (chip P0 power downclock to
    ~2.0GHz under sustained full-chip load; pure-PE microbenches run at
    216ns but PE+DMA+CC tips the power budget).  The schedule aims for a
    gapless PE stream rather than fighting the clock.
  - x is split between the sync queue (even 2-chunk pieces interleaved
    between sweep-A strip batches, pacing supply to demand) and the
    gpsimd queue (odd pieces as 4 strided 1MB DMAs) so the early window
    is not bandwidth-oversubscribed the way eager front-loading was.
  - t_sb gather loads ride the gpsimd queue h-half-major right behind
    the next AG trigger they cannot delay.
  - out stores alternate sync/gpsimd so the ACT queue never carries a
    store between epilogue relus.

DMA discipline: DMA_DIRECT2D executes synchronously on the issuing
engine's queue AND all queues share one small DMA-completion semaphore
pool, so a blocked DMA stalls unrelated queues.  Sync: strip loads,
interleaved x, t_in stores, half the out stores.  Activation: w + bias
seed only.  GpSimd: odd x pieces (head), AG triggers + AG-gated t_sb
loads (middle), half the out stores (tail).

All big operands are host-cast to bf16 (full PE rate) with fp32 PSUM
accumulation; s-matrices are host-transposed so the contraction dim lands
on SBUF partitions with contiguous DMAs.  x needs no transpose in this
formulation.
"""

import os

import numpy as np

N_CORES = 8
N = 8192
K = 2048
NK = N - K          # 6144
D = 512
HD = D // 2         # 256 (narrow-sweep width)
ROWS = N // N_CORES  # 1024 rows per core
P = 128
RCH = ROWS // P      # 8  (row chunks per core / output strips)
NCH = N // P         # 64 (contraction chunks over full N)
KCH = K // P         # 16 (low-band chunks; high band = NCH - KCH = 48)
DCH = D // P         # 4  (depth chunks)
NSUB = 4             # sub-AllGathers for t
SUBR = ROWS // NSUB  # 256 rows per rank per sub-AG
SB = 4               # n-chunks per sweep-strip DMA batch
NB = NCH // SB       # 16 strip batches per sweep
NWARM = 8            # PE warm-up junk matmuls

DEBUG = os.environ.get("SPGAT_DEBUG", "0") == "1"

_CACHE = {}

# t-chunk arrival order: sub-AG g delivers, for every rank c, t rows
# [1024c + 256g, 1024c + 256(g+1)) = global chunks 8c + 2g + {0,1}.
# Gathers trigger in row order 0,1,2,3 now, so consume in that order.
GORDER = [0, 1, 2, 3]
PAIRS = [(8 * c + 2 * g, g, c) for g in GORDER for c in range(N_CORES)]


def _build_nc():
    import concourse.mybir as mybir
    import concourse.tile as tile
    from concourse import bacc

    f32 = mybir.dt.float32
    bf16 = mybir.dt.bfloat16
    cdt = bf16

    nc = bacc.Bacc(
        "TRN2", target_bir_lowering=False, debug=False, num_devices=N_CORES
    )

    # all big operands host-packed so every DMA reads 2KB+ contiguous
    # per-partition lines (max descriptor efficiency)
    xp = nc.dram_tensor("xp", [NCH // 2, P, 2 * D], cdt,
                        kind="ExternalInput").ap()
    w = nc.dram_tensor("w", [D, D], cdt, kind="ExternalInput").ap()
    seedb = nc.dram_tensor("seedb", [P, D], f32, kind="ExternalInput").ap()
    stpA = nc.dram_tensor("stpA", [NB, P, SB * D], cdt,
                          kind="ExternalInput").ap()
    stpB = nc.dram_tensor("stpB", [2, NB, P, SB * HD], cdt,
                          kind="ExternalInput").ap()
    s0p = nc.dram_tensor("s0p", [KCH // 2, P, 2 * ROWS], cdt,
                         kind="ExternalInput").ap()
    s2p = nc.dram_tensor("s2p", [(NCH - KCH) // 2, P, 2 * ROWS], cdt,
                         kind="ExternalInput").ap()
    out = nc.dram_tensor("out", [ROWS, D], f32, kind="ExternalOutput").ap()
    if DEBUG:
        t_dump = nc.dram_tensor("t_dump", [N, D], cdt, kind="ExternalOutput").ap()

    groups = [list(range(N_CORES))]

    with tile.TileContext(nc) as tc:
        with (
            tc.tile_pool(name="const", bufs=1) as const,
            tc.tile_pool(name="bigA", bufs=1) as bigA,
            tc.tile_pool(name="bigB", bufs=1) as bigB,
            tc.tile_pool(name="strips1", bufs=4) as strips1,
            tc.tile_pool(name="strips3", bufs=5) as strips3,
            tc.tile_pool(name="stage", bufs=4) as stage,
            tc.tile_pool(name="epi", bufs=2) as epi,
            tc.tile_pool(name="stash", bufs=1) as stashp,
            tc.tile_pool(name="ps", bufs=8, space="PSUM") as ps,
            tc.tile_pool(name="dram", bufs=1, space="DRAM") as dram,
        ):
            # ---- collective warm-up: absorb CC startup + skew.  No input
            # deps so the trigger is the very first gpsimd instruction.
            warm_in = dram.tile([8, 8], f32, name="warm_in")
            warm_out = dram.tile([64, 8], f32, name="warm_out", addr_space="Shared")
            nc.gpsimd.collective_compute(
                "AllGather",
                mybir.AluOpType.bypass,
                replica_groups=groups,
                ins=[warm_in.opt()],
                outs=[warm_out.opt()],
            )

            # ---- PSUM allocation order fixes the 8-bank tag rotation:
            # accsA 0-3, accsB1 4-7, tpsA 0-3, accsB2 4-7, tpsB1 0-1,
            # tpsB2 2-3, accs3 4-7 then 0-3.
            accsA = [ps.tile([P, D], f32, name=f"uA_{dc}", tag="acc")
                     for dc in range(DCH)]
            accsB1 = [ps.tile([P, HD], f32, name=f"uB1_{dc}", tag="acc")
                      for dc in range(DCH)]

            # ---- PE warm-up junk matmuls (HAM activity) on memset tiles;
            # they accumulate into accsA[0] with start=True and the first
            # real matmul (j == 0, start=True) resets the bank.
            wj = const.tile([P, P], cdt, name="wj")
            mj = const.tile([P, D], cdt, name="mj")
            nc.vector.memset(wj[:], 0.0)
            nc.vector.memset(mj[:], 0.0)
            for _ in range(NWARM):
                nc.tensor.matmul(accsA[0][:], wj[:], mj[:], start=True, stop=True)

            # ---- head DMAs.  Sync: x chunks 0,1 singly (j=0 matmuls fire
            # asap).  GpSimd: odd x pieces as 4 strided 1MB DMAs.  Scalar:
            # w + bias seed.
            x_sb = bigA.tile([P, NCH, D], cdt, name="x_sb", tag="bigA")
            nc.sync.dma_start(x_sb[:, 0, :], xp[0][:, :D])
            nc.sync.dma_start(x_sb[:, 1, :], xp[0][:, D:])
            for m in range(4):
                # odd pieces 1+8m..7+8m -> chunks 16m + {2,3, 6,7, 10,11, 14,15}
                nc.gpsimd.dma_start(
                    x_sb[:, 16 * m : 16 * (m + 1), :]
                    .rearrange("p (k f) d -> p k (f d)", f=4)[:, :, 2 * D : 4 * D],
                    xp[1 + 8 * m : 8 * (m + 1) : 2].rearrange("b p a -> p b a"),
                )
            w_sb = const.tile([P, DCH, D], cdt, name="w_sb")
            nc.scalar.dma_start(w_sb[:], w.rearrange("(c p) d -> p c d", p=P))
            bsb = const.tile([P, D], f32, name="bsb")
            nc.scalar.dma_start(bsb[:], seedb[:])

            # ---- steps 1+2 infrastructure
            t_in = dram.tile([ROWS, D], cdt, name="t_in")
            t_outs = [
                dram.tile([SUBR * N_CORES, D], cdt, name=f"t_out{g}",
                          addr_space="Shared")
                for g in range(NSUB)
            ]

            def t_subag(g):
                nc.gpsimd.collective_compute(
                    "AllGather",
                    mybir.AluOpType.bypass,
                    replica_groups=groups,
                    ins=[t_in[SUBR * g : SUBR * (g + 1), :].opt()],
                    outs=[t_outs[g].opt()],
                )

            utA = const.tile([P, DCH, D], cdt, name="utA")
            utB = [const.tile([P, DCH, HD], cdt, name=f"utB{s}")
                   for s in range(2)]

            def sweepA():
                """512-wide sweep, even x pieces interleaved on sync."""
                for bk in range(NB):
                    strip = strips1.tile([P, SB, D], cdt, name=f"sA_{bk}",
                                         tag="strip")
                    if bk == 0:
                        for j2 in range(SB):
                            nc.sync.dma_start(
                                strip[:, j2, :],
                                stpA[bk][:, D * j2 : D * (j2 + 1)],
                            )
                    else:
                        nc.sync.dma_start(
                            strip[:].rearrange("p a b -> p (a b)"), stpA[bk]
                        )
                    if bk <= 14:
                        pc = 2 * bk + 2  # even piece -> chunks 4bk+4, 4bk+5
                        nc.sync.dma_start(
                            x_sb[:, 2 * pc : 2 * pc + 2, :]
                            .rearrange("p c d -> p (c d)"),
                            xp[pc],
                        )
                    for j2 in range(SB):
                        j = SB * bk + j2
                        for dc in range(DCH):
                            nc.tensor.matmul(
                                accsA[dc][:],
                                x_sb[:, j, P * dc : P * (dc + 1)],
                                strip[:, j2, :],
                                start=(j == 0),
                                stop=(j == NCH - 1),
                            )

            def sweepB(s, accs, b0, b1):
                """256-wide sweep s (0 -> i 512..767, 1 -> i 768..1023)."""
                for bk in range(b0, b1):
                    strip = strips1.tile([P, SB, HD], cdt, name=f"sB{s}_{bk}",
                                         tag="strip")
                    nc.sync.dma_start(
                        strip[:].rearrange("p a b -> p (a b)"), stpB[s, bk]
                    )
                    for j2 in range(SB):
                        j = SB * bk + j2
                        for dc in range(DCH):
                            nc.tensor.matmul(
                                accs[dc][:],
                                x_sb[:, j, P * dc : P * (dc + 1)],
                                strip[:, j2, :],
                                start=(j == 0),
                                stop=(j == NCH - 1),
                            )

            def drain(ut, accs, width):
                for dc in range(DCH):
                    nc.vector.tensor_copy(ut[:, dc, :width], accs[dc][:])

            def step2(ut, row0, nblk, tps, ag_map):
                """stage t rows [row0, row0+128*nblk); trigger AGs per
                ag_map {local_ib: g} after that block's staging store."""
                for ib in range(nblk):
                    tp = tps[ib]
                    for dc in range(DCH):
                        nc.tensor.matmul(
                            tp[:],
                            ut[:, dc, P * ib : P * (ib + 1)],
                            w_sb[:, dc, :],
                            start=(dc == 0),
                            stop=(dc == DCH - 1),
                        )
                    tst = stage.tile([P, D], cdt, name=f"t_st_{row0}_{ib}",
                                     tag="st")
                    nc.vector.tensor_copy(tst[:], tp[:])
                    r0 = row0 + P * ib
                    nc.sync.dma_start(t_in[r0 : r0 + P, :], tst[:])
                    if ib in ag_map:
                        t_subag(ag_map[ib])

            t_sb = bigB.tile([P, NCH, D], cdt, name="t_sb", tag="bigB")
            t_sb_r = t_sb[:].rearrange("p (c r) d -> p r c d", r=8)

            def t_loads(g):
                """gather consumption loads for group g, h-half-major so
                the first consumed pair lands after two loads."""
                for h in range(2):
                    for u in range(2):
                        nc.gpsimd.dma_start(
                            t_sb_r[:, 2 * g + u, 4 * h : 4 * h + 4, :],
                            t_outs[g].rearrange(
                                "(c q p) d -> p c q d", p=P, q=2
                            )[:, 4 * h : 4 * h + 4, u, :],
                        )

            # ---- the PE stream
            sweepA()
            drain(utA, accsA, D)
            sweepB(0, accsB1, 0, 2)  # hide drain latency
            tpsA = [ps.tile([P, D], f32, name=f"tpA_{i}", tag="acc")
                    for i in range(4)]
            step2(utA, 0, 4, tpsA, {1: 0, 3: 1})
            t_loads(0)
            sweepB(0, accsB1, 2, NB)
            drain(utB[0], accsB1, HD)
            accsB2 = [ps.tile([P, HD], f32, name=f"uB2_{dc}", tag="acc")
                      for dc in range(DCH)]
            sweepB(1, accsB2, 0, 2)
            tpsB1 = [ps.tile([P, D], f32, name=f"tpB1_{i}", tag="acc")
                     for i in range(2)]
            step2(utB[0], 2 * SUBR, 2, tpsB1, {1: 2})
            t_loads(1)
            sweepB(1, accsB2, 2, NB)
            drain(utB[1], accsB2, HD)

            tpsB2 = [ps.tile([P, D], f32, name=f"tpB2_{i}", tag="acc")
                     for i in range(2)]
            accs3 = [
                ps.tile([P, D], f32, name=f"acc3_{nt}", tag="acc")
                for nt in range(RCH)
            ]
            # accs3[0..3] sit on accsB2's banks (free after its drain);
            # seed them before step2B2 so only 4..7 wait on its staging.
            for nt in range(4):
                nc.vector.tensor_copy(accs3[nt][:], bsb[:])
            step2(utB[1], 3 * SUBR, 2, tpsB2, {1: 3})
            t_loads(2)
            t_loads(3)
            for nt in range(4, RCH):
                nc.scalar.copy(accs3[nt][:], bsb[:])

            if DEBUG:
                for j in range(NCH):
                    nc.sync.dma_start(
                        t_dump[P * j : P * (j + 1), :], t_sb[:, j, :]
                    )

            # ---- phase 3
            HI_PAIRS = [e for e in PAIRS if e[0] >= KCH]
            LO_PAIRS = [e for e in PAIRS if e[0] < KCH]
            stash = [
                stashp.tile([P, D], f32, name=f"hst_{nt}", tag=f"hst{nt}")
                for nt in range(RCH)
            ]
            for idx, (j, g, c) in enumerate(HI_PAIRS):
                jj = j - KCH
                strip = strips3.tile([P, 2, ROWS], cdt, name=f"rh_{j}",
                                     tag="strip3")
                nc.sync.dma_start(
                    strip[:].rearrange("p a b -> p (a b)"), s2p[jj // 2]
                )
                for u in range(2):
                    for nt in range(RCH):
                        nc.tensor.matmul(
                            accs3[nt][:],
                            strip[:, u, P * nt : P * (nt + 1)],
                            t_sb[:, j + u, :],
                            start=False,
                            stop=(idx == len(HI_PAIRS) - 1 and u == 1),
                        )
            # stash = acc (= hi + bias; softmax folded into s2 on host),
            # then re-seed for the low band right behind the stash read
            for nt in range(4):
                nc.vector.tensor_copy(stash[nt][:], accs3[nt][:])
                nc.vector.tensor_copy(accs3[nt][:], bsb[:])
            for nt in range(4, RCH):
                nc.scalar.copy(stash[nt][:], accs3[nt][:])
                nc.scalar.copy(accs3[nt][:], bsb[:])
            # relu the stash in place (during the low band):
            # relu(max(u,v)) == max(relu(u), relu(v))
            for nt in range(RCH):
                nc.scalar.activation(
                    stash[nt][:], stash[nt][:],
                    mybir.ActivationFunctionType.Relu,
                )

            # low band, all strips except the last two: strip-major
            for j, g, c in LO_PAIRS[:-2]:
                strip = strips3.tile([P, 2, ROWS], cdt, name=f"rl_{j}",
                                     tag="strip3")
                nc.sync.dma_start(
                    strip[:].rearrange("p a b -> p (a b)"), s0p[j // 2]
                )
                for u in range(2):
                    for nt in range(RCH):
                        nc.tensor.matmul(
                            accs3[nt][:],
                            strip[:, u, P * nt : P * (nt + 1)],
                            t_sb[:, j + u, :],
                            start=False,
                            stop=False,
                        )
            # last two strips nt-major: accumulators complete staggered so
            # the relu/max/store epilogue pipelines behind the matmuls
            tail = []
            for j, g, c in LO_PAIRS[-2:]:
                strip = strips3.tile([P, 2, ROWS], cdt, name=f"rt_{j}",
                                     tag="strip3")
                nc.sync.dma_start(
                    strip[:].rearrange("p a b -> p (a b)"), s0p[j // 2]
                )
                tail.append((j, strip))
            for nt in range(RCH):
                for ti, (j, strip) in enumerate(tail):
                    for u in range(2):
                        nc.tensor.matmul(
                            accs3[nt][:],
                            strip[:, u, P * nt : P * (nt + 1)],
                            t_sb[:, j + u, :],
                            start=False,
                            stop=(ti == len(tail) - 1 and u == 1),
                        )
                lo = epi.tile([P, D], f32, name=f"elo_{nt}", tag="elo")
                nc.scalar.activation(
                    lo[:], accs3[nt][:], mybir.ActivationFunctionType.Relu,
                )
                osb = epi.tile([P, D], f32, name=f"osb_{nt}", tag="osb")
                nc.vector.tensor_tensor(
                    osb[:], lo[:], stash[nt][:], mybir.AluOpType.max
                )
                row0 = P * nt
                eng = nc.sync if nt % 2 == 0 else nc.gpsimd
                eng.dma_start(out[row0 : row0 + P, :], osb[:])

    nc.compile()
    return nc


def _get_nc():
    if "nc" not in _CACHE:
        _CACHE["nc"] = _build_nc()
    return _CACHE["nc"]


def _shard_inputs(x, weights, alpha, bias, s0, s1, s2, s3):
    import ml_dtypes

    cnp = ml_dtypes.bfloat16

    def prep(a, scale=None):  # transpose (+ scale) + cast, C-contiguous
        t = a.T if scale is None else a.T * scale
        return np.ascontiguousarray(t).astype(cnp, copy=False)

    # softmax(alpha) folded into the low/high band matrices host-side
    af = np.asarray(alpha, dtype=np.float64)
    e = np.exp(af - af.max())
    a_sm = (e / e.sum()).astype(np.float32)

    seedb = np.ascontiguousarray(
        np.broadcast_to(np.asarray(bias, dtype=np.float32)[None, :], (P, D))
    )
    w_p = np.ascontiguousarray(weights).astype(cnp, copy=False)
    # xp[b, p, (j2 d)] = x[128*(2b+j2)+p, d]: 2KB contiguous per partition
    x_p = np.ascontiguousarray(
        x.astype(cnp, copy=False).reshape(NCH // 2, 2, P, D).transpose(0, 2, 1, 3)
        .reshape(NCH // 2, P, 2 * D)
    )

    def pack_A(t):  # t: [n, 1024] -> sweep A (i 0..511): [NB, P, SB*D]
        a = t[:, :D].reshape(NB, SB, P, D).transpose(0, 2, 1, 3)
        return np.ascontiguousarray(a.reshape(NB, P, SB * D))

    def pack_B(t):  # sweeps B1/B2 (i 512..767, 768..1023): [2, NB, P, SB*HD]
        a = t[:, D:].reshape(NB, SB, P, 2, HD).transpose(3, 0, 2, 1, 4)
        return np.ascontiguousarray(a.reshape(2, NB, P, SB * HD))

    def pack_pairs(t):  # t: [n, i] -> [n/256, P, 2*i]
        n, i = t.shape
        a = t.reshape(n // (2 * P), 2, P, i).transpose(0, 2, 1, 3)
        return np.ascontiguousarray(a.reshape(n // (2 * P), P, 2 * i))

    in_maps = []
    for c in range(N_CORES):
        r0, r1 = ROWS * c, ROWS * (c + 1)
        # S = concat(s1, s3) rows; core c owns rows [r0, r1)
        if r1 <= K:
            s_rows = s1[r0:r1]
        elif r0 >= K:
            s_rows = s3[r0 - K : r1 - K]
        else:  # straddles the boundary (not the case for these shapes)
            s_rows = np.concatenate([s1[r0:], s3[: r1 - K]], axis=0)
        st = prep(s_rows)
        in_maps.append(
            {
                "xp": x_p,
                "w": w_p,
                "seedb": seedb,
                "stpA": pack_A(st),
                "stpB": pack_B(st),
                "s0p": pack_pairs(prep(s0[r0:r1], a_sm[0])),
                "s2p": pack_pairs(prep(s2[r0:r1], a_sm[1])),
            }
        )
    return in_maps


def kernel(x, weights, alpha, bias, s0, s1, s2, s3, _trace=False):
    from concourse.bass_utils import run_bass_kernel_spmd

    nc = _get_nc()
    in_maps = _shard_inputs(
        np.asarray(x), np.asarray(weights), np.asarray(alpha), np.asarray(bias),
        np.asarray(s0), np.asarray(s1), np.asarray(s2), np.asarray(s3),
    )
    kwargs = {}
    if _trace:
        run_bass_kernel_spmd(nc, in_maps, core_ids=list(range(N_CORES)))
        kwargs = dict(trace=True, trace_cores=list(range(N_CORES)))
    r = run_bass_kernel_spmd(nc, in_maps, core_ids=list(range(N_CORES)), **kwargs)
    full = np.concatenate([res["out"] for res in r.results], axis=0)
    if _trace:
        return full, r
    return full


# revision 15
# speedup vs baseline: 1.0750x; 1.0750x over previous
"""SpGAT_Conv Trainium2 kernel: 8-core SPMD spectral GNN conv.

Math (reference):
    a = softmax(alpha)
    pre = x @ W                                   [N, D]
    out_low  = s0 @ (a0 * (s1 @ pre))             [N, D]
    out_high = s2 @ (a1 * (s3 @ pre))             [N, D]
    out = relu(max(out_low, out_high) + bias)

Re-association: t = S @ (x @ W) == (S @ x) @ W with S = concat(s1, s3).
Row-sharding t's rows across 8 cores makes the x@W work perfectly sharded
too (it rides on each core's own 1024 rows of u = S_c @ x), cutting
per-core PE work to the distribution-optimal 1056 big-matmul equivalents.

    step 1: u_c^T = x^T S_c^T accumulated over n-chunks; stationary = x
            chunks (natural layout), moving = S_c^T strips.  One 512-wide
            sweep (A, i-cols 0..511) then two 256-wide sweeps (B1, B2) —
            narrower late sweeps stage t earlier so the AllGather chain
            (whose CC engine only comes alive ~60-85us in) finishes well
            before phase 3 needs its data.
    step 2: t rows = u^T slices stationary, W moving; each sweep's rows
            are staged + AllGathered immediately (AG0/AG1 after sweep A,
            AG2 after B1, AG3 after B2).
    phase 3: out_c = relu(max(s0_c@t1 + bias, s2_c@t3 + bias)), high band
             then low band in AG-arrival order g 0,1,2,3.  softmax(alpha)
             is folded into s0/s2 on the host, bias is passed
             pre-broadcast [128, D] and seeded into PSUM, so the epilogue
             per strip is one ACT relu + one DVE max.  The last two
             low-band strips run nt-major so the 8 accumulators complete
             staggered and the epilogue pipelines behind the matmuls.

Schedule notes (v3):
  - PE warm-up junk matmuls keep the HAM activity window busy from
    ~6.5us so the first real matmuls aren't clock-gated.
  - Steady-state matmul rate is ~263ns (chip P0 power downclock to
    ~2.0GHz under sustained full-chip load; pure-PE microbenches run at
    216ns but PE+DMA+CC tips the power budget).  The schedule aims for a
    gapless PE stream rather than fighting the clock.
  - x is split between the sync queue (even 2-chunk pieces interleaved
    between sweep-A strip batches, pacing supply to demand) and the
    gpsimd queue (odd pieces as 4 strided 1MB DMAs) so the early window
    is not bandwidth-oversubscribed the way eager front-loading was.
  - t_sb gather loads ride the gpsimd queue h-half-major right behind
    the next AG trigger they cannot delay.
  - out stores alternate sync/gpsimd so the ACT queue never carries a
    store between epilogue relus.

DMA discipline: DMA_DIRECT2D executes synchronously on the issuing
engine's queue AND all queues share one small DMA-completion semaphore
pool, so a blocked DMA stalls unrelated queues.  Sync: strip loads,
interleaved x, t_in stores, half the out stores.  Activation: w + bias
seed only.  GpSimd: odd x pieces (head), AG triggers + AG-gated t_sb
loads (middle), half the out stores (tail).

All big operands are host-cast to bf16 (full PE rate) with fp32 PSUM
accumulation; s-matrices are host-transposed so the contraction dim lands
on SBUF partitions with contiguous DMAs.  x needs no transpose in this
formulation.
"""

import os

import numpy as np

N_CORES = 8
N = 8192
K = 2048
NK = N - K          # 6144
D = 512
HD = D // 2         # 256 (narrow-sweep width)
ROWS = N // N_CORES  # 1024 rows per core
P = 128
RCH = ROWS // P      # 8  (row chunks per core / output strips)
NCH = N // P         # 64 (contraction chunks over full N)
KCH = K // P         # 16 (low-band chunks; high band = NCH - KCH = 48)
DCH = D // P         # 4  (depth chunks)
NSUB = 4             # sub-AllGathers for t
SUBR = ROWS // NSUB  # 256 rows per rank per sub-AG
SB = 4               # n-chunks per sweep-strip DMA batch
NB = NCH // SB       # 16 strip batches per sweep
NWARM = 8            # PE warm-up junk matmuls

DEBUG = os.environ.get("SPGAT_DEBUG", "0") == "1"

_CACHE = {}

# t-chunk arrival order: sub-AG g delivers, for every rank c, t rows
# [1024c + 256g, 1024c + 256(g+1)) = global chunks 8c + 2g + {0,1}.
# Gathers trigger in row order 0,1,2,3 now, so consume in that order.
GORDER = [0, 1, 2, 3]
PAIRS = [(8 * c + 2 * g, g, c) for g in GORDER for c in range(N_CORES)]


def _build_nc():
    import concourse.mybir as mybir
    import concourse.tile as tile
    from concourse import bacc

    f32 = mybir.dt.float32
    bf16 = mybir.dt.bfloat16
    cdt = bf16

    nc = bacc.Bacc(
        "TRN2", target_bir_lowering=False, debug=False, num_devices=N_CORES
    )

    # all big operands host-packed so every DMA reads 2KB+ contiguous
    # per-partition lines (max descriptor efficiency)
    xp = nc.dram_tensor("xp", [NCH // 2, P, 2 * D], cdt,
                        kind="ExternalInput").ap()
    w = nc.dram_tensor("w", [D, D], cdt, kind="ExternalInput").ap()
    seedb = nc.dram_tensor("seedb", [P, D], f32, kind="ExternalInput").ap()
    stpA = nc.dram_tensor("stpA", [NB, P, SB * D], cdt,
                          kind="ExternalInput").ap()
    stpB = nc.dram_tensor("stpB", [2, NB, P, SB * HD], cdt,
                          kind="ExternalInput").ap()
    s0p = nc.dram_tensor("s0p", [KCH // 2, P, 2 * ROWS], cdt,
                         kind="ExternalInput").ap()
    s2p = nc.dram_tensor("s2p", [(NCH - KCH) // 2, P, 2 * ROWS], cdt,
                         kind="ExternalInput").ap()
    out = nc.dram_tensor("out", [ROWS, D], f32, kind="ExternalOutput").ap()
    if DEBUG:
        t_dump = nc.dram_tensor("t_dump", [N, D], cdt, kind="ExternalOutput").ap()

    groups = [list(range(N_CORES))]

    with tile.TileContext(nc) as tc:
        with (
            tc.tile_pool(name="const", bufs=1) as const,
            tc.tile_pool(name="bigA", bufs=1) as bigA,
            tc.tile_pool(name="bigB", bufs=1) as bigB,
            tc.tile_pool(name="strips1", bufs=4) as strips1,
            tc.tile_pool(name="strips3", bufs=4) as strips3,
            tc.tile_pool(name="stage", bufs=4) as stage,
            tc.tile_pool(name="epi", bufs=3) as epi,
            tc.tile_pool(name="stash", bufs=1) as stashp,
            tc.tile_pool(name="ps", bufs=8, space="PSUM") as ps,
            tc.tile_pool(name="dram", bufs=1, space="DRAM") as dram,
        ):
            # ---- collective warm-up: absorb CC startup + skew.  No input
            # deps so the trigger is the very first gpsimd instruction.
            warm_in = dram.tile([8, 8], f32, name="warm_in")
            warm_out = dram.tile([64, 8], f32, name="warm_out", addr_space="Shared")
            nc.gpsimd.collective_compute(
                "AllGather",
                mybir.AluOpType.bypass,
                replica_groups=groups,
                ins=[warm_in.opt()],
                outs=[warm_out.opt()],
            )

            # ---- PSUM allocation order fixes the 8-bank tag rotation:
            # accsA 0-3, accsB1 4-7, tpsA 0-3, accsB2 4-7, tpsB1 0-1,
            # tpsB2 2-3, accs3 4-7 then 0-3.
            accsA = [ps.tile([P, D], f32, name=f"uA_{dc}", tag="acc")
                     for dc in range(DCH)]
            accsB1 = [ps.tile([P, HD], f32, name=f"uB1_{dc}", tag="acc")
                      for dc in range(DCH)]

            # ---- PE warm-up junk matmuls (HAM activity) on memset tiles;
            # they accumulate into accsA[0] with start=True and the first
            # real matmul (j == 0, start=True) resets the bank.
            wj = const.tile([P, P], cdt, name="wj")
            mj = const.tile([P, D], cdt, name="mj")
            nc.vector.memset(wj[:], 0.0)
            nc.vector.memset(mj[:], 0.0)
            for _ in range(NWARM):
                nc.tensor.matmul(accsA[0][:], wj[:], mj[:], start=True, stop=True)

            # ---- head DMAs.  Sync: x chunks 0,1 singly (j=0 matmuls fire
            # asap).  GpSimd: odd x pieces as 4 strided 1MB DMAs.  Scalar:
            # w + bias seed.
            x_sb = bigA.tile([P, NCH, D], cdt, name="x_sb", tag="bigA")
            nc.sync.dma_start(x_sb[:, 0, :], xp[0][:, :D])
            nc.sync.dma_start(x_sb[:, 1, :], xp[0][:, D:])
            # remaining x as small pair-pieces on the scalar queue: fine
            # interleaving with the sync strip stream at the HBM level, no
            # multi-us block DMA ever monopolizes the fabric
            for pc in range(1, NCH // 2):
                nc.scalar.dma_start(
                    x_sb[:, 2 * pc : 2 * pc + 2, :]
                    .rearrange("p c d -> p (c d)"),
                    xp[pc],
                )
            w_sb = const.tile([P, DCH, D], cdt, name="w_sb")
            nc.scalar.dma_start(w_sb[:], w.rearrange("(c p) d -> p c d", p=P))
            bsb = const.tile([P, D], f32, name="bsb")
            nc.scalar.dma_start(bsb[:], seedb[:])

            # ---- steps 1+2 infrastructure
            t_in = dram.tile([ROWS, D], cdt, name="t_in")
            t_outs = [
                dram.tile([SUBR * N_CORES, D], cdt, name=f"t_out{g}",
                          addr_space="Shared")
                for g in range(NSUB)
            ]

            def t_subag(g):
                nc.gpsimd.collective_compute(
                    "AllGather",
                    mybir.AluOpType.bypass,
                    replica_groups=groups,
                    ins=[t_in[SUBR * g : SUBR * (g + 1), :].opt()],
                    outs=[t_outs[g].opt()],
                )

            utA = const.tile([P, DCH, D], cdt, name="utA")
            utB = [const.tile([P, DCH, HD], cdt, name=f"utB{s}")
                   for s in range(2)]

            def sweepA():
                """512-wide sweep, even x pieces interleaved on sync."""
                for bk in range(NB):
                    strip = strips1.tile([P, SB, D], cdt, name=f"sA_{bk}",
                                         tag="strip")
                    if bk == 0:
                        for j2 in range(SB):
                            nc.sync.dma_start(
                                strip[:, j2, :],
                                stpA[bk][:, D * j2 : D * (j2 + 1)],
                            )
                    else:
                        nc.sync.dma_start(
                            strip[:].rearrange("p a b -> p (a b)"), stpA[bk]
                        )
                    for j2 in range(SB):
                        j = SB * bk + j2
                        for dc in range(DCH):
                            nc.tensor.matmul(
                                accsA[dc][:],
                                x_sb[:, j, P * dc : P * (dc + 1)],
                                strip[:, j2, :],
                                start=(j == 0),
                                stop=(j == NCH - 1),
                            )

            def sweepB(s, accs, b0, b1):
                """256-wide sweep s (0 -> i 512..767, 1 -> i 768..1023)."""
                for bk in range(b0, b1):
                    strip = strips1.tile([P, SB, HD], cdt, name=f"sB{s}_{bk}",
                                         tag="strip")
                    nc.sync.dma_start(
                        strip[:].rearrange("p a b -> p (a b)"), stpB[s, bk]
                    )
                    for j2 in range(SB):
                        j = SB * bk + j2
                        for dc in range(DCH):
                            nc.tensor.matmul(
                                accs[dc][:],
                                x_sb[:, j, P * dc : P * (dc + 1)],
                                strip[:, j2, :],
                                start=(j == 0),
                                stop=(j == NCH - 1),
                            )

            def drain(ut, accs, width):
                # split across DVE and ACT so the PSUM banks free ~2x faster
                for dc in range(2):
                    nc.vector.tensor_copy(ut[:, dc, :width], accs[dc][:])
                for dc in range(2, DCH):
                    nc.scalar.copy(ut[:, dc, :width], accs[dc][:])

            def step2(ut, row0, nblk, tps, ag_map):
                """stage t rows [row0, row0+128*nblk); trigger AGs per
                ag_map {local_ib: g} after that block's staging store."""
                for ib in range(nblk):
                    tp = tps[ib]
                    for dc in range(DCH):
                        nc.tensor.matmul(
                            tp[:],
                            ut[:, dc, P * ib : P * (ib + 1)],
                            w_sb[:, dc, :],
                            start=(dc == 0),
                            stop=(dc == DCH - 1),
                        )
                    tst = stage.tile([P, D], cdt, name=f"t_st_{row0}_{ib}",
                                     tag="st")
                    nc.vector.tensor_copy(tst[:], tp[:])
                    r0 = row0 + P * ib
                    nc.sync.dma_start(t_in[r0 : r0 + P, :], tst[:])
                    if ib in ag_map:
                        t_subag(ag_map[ib])

            t_sb = bigB.tile([P, NCH, D], cdt, name="t_sb", tag="bigB")
            t_sb_r = t_sb[:].rearrange("p (c r) d -> p r c d", r=8)

            def t_loads(g):
                """gather consumption loads for group g, h-half-major so
                the first consumed pair lands after two loads."""
                for h in range(2):
                    for u in range(2):
                        nc.gpsimd.dma_start(
                            t_sb_r[:, 2 * g + u, 4 * h : 4 * h + 4, :],
                            t_outs[g].rearrange(
                                "(c q p) d -> p c q d", p=P, q=2
                            )[:, 4 * h : 4 * h + 4, u, :],
                        )

            # ---- the PE stream
            sweepA()
            drain(utA, accsA, D)
            sweepB(0, accsB1, 0, 2)  # hide drain latency
            tpsA = [ps.tile([P, D], f32, name=f"tpA_{i}", tag="acc")
                    for i in range(4)]
            step2(utA, 0, 4, tpsA, {1: 0, 3: 1})
            t_loads(0)
            sweepB(0, accsB1, 2, NB)
            drain(utB[0], accsB1, HD)
            # PSUM rotation: tpsB1/tpsB2 allocated first land on accsB1's
            # banks (4-7, free after its drain); accsB2 then lands on
            # tpsA's banks (0-3, free since ~step2A) so sweep B2's head
            # overlaps B1's drain instead of serializing behind it.
            tpsB1 = [ps.tile([P, D], f32, name=f"tpB1_{i}", tag="acc")
                     for i in range(2)]
            tpsB2 = [ps.tile([P, D], f32, name=f"tpB2_{i}", tag="acc")
                     for i in range(2)]
            accsB2 = [ps.tile([P, HD], f32, name=f"uB2_{dc}", tag="acc")
                      for dc in range(DCH)]
            sweepB(1, accsB2, 0, 2)
            step2(utB[0], 2 * SUBR, 2, tpsB1, {1: 2})
            t_loads(1)
            sweepB(1, accsB2, 2, NB)
            drain(utB[1], accsB2, HD)

            accs3 = [
                ps.tile([P, D], f32, name=f"acc3_{nt}", tag="acc")
                for nt in range(RCH)
            ]
            # accs3[0,1] sit on tpsB1's banks (free after its staging);
            # seed them before step2B2 so they don't wait on its staging.
            for nt in range(2):
                nc.vector.tensor_copy(accs3[nt][:], bsb[:])
            step2(utB[1], 3 * SUBR, 2, tpsB2, {1: 3})
            t_loads(2)
            t_loads(3)
            for nt in range(2, 5):
                nc.vector.tensor_copy(accs3[nt][:], bsb[:])
            for nt in range(5, RCH):
                nc.scalar.copy(accs3[nt][:], bsb[:])

            if DEBUG:
                for j in range(NCH):
                    nc.sync.dma_start(
                        t_dump[P * j : P * (j + 1), :], t_sb[:, j, :]
                    )

            # ---- phase 3
            HI_PAIRS = [e for e in PAIRS if e[0] >= KCH]
            LO_PAIRS = [e for e in PAIRS if e[0] < KCH]
            stash = [
                stashp.tile([P, D], f32, name=f"hst_{nt}", tag=f"hst{nt}")
                for nt in range(RCH)
            ]
            for idx, (j, g, c) in enumerate(HI_PAIRS):
                jj = j - KCH
                strip = strips3.tile([P, 2, ROWS], cdt, name=f"rh_{j}",
                                     tag="strip3")
                nc.sync.dma_start(
                    strip[:].rearrange("p a b -> p (a b)"), s2p[jj // 2]
                )
                for u in range(2):
                    for nt in range(RCH):
                        nc.tensor.matmul(
                            accs3[nt][:],
                            strip[:, u, P * nt : P * (nt + 1)],
                            t_sb[:, j + u, :],
                            start=False,
                            stop=(idx == len(HI_PAIRS) - 1 and u == 1),
                        )
            # stash = acc (= hi + bias; softmax folded into s2 on host),
            # then re-seed for the low band right behind the stash read
            for nt in range(4):
                nc.vector.tensor_copy(stash[nt][:], accs3[nt][:])
                nc.vector.tensor_copy(accs3[nt][:], bsb[:])
            for nt in range(4, RCH):
                nc.scalar.copy(stash[nt][:], accs3[nt][:])
                nc.scalar.copy(accs3[nt][:], bsb[:])
            # relu the stash in place (during the low band):
            # relu(max(u,v)) == max(relu(u), relu(v))
            for nt in range(RCH):
                nc.scalar.activation(
                    stash[nt][:], stash[nt][:],
                    mybir.ActivationFunctionType.Relu,
                )

            # low band, all strips except the last two: strip-major
            for j, g, c in LO_PAIRS[:-2]:
                strip = strips3.tile([P, 2, ROWS], cdt, name=f"rl_{j}",
                                     tag="strip3")
                nc.sync.dma_start(
                    strip[:].rearrange("p a b -> p (a b)"), s0p[j // 2]
                )
                for u in range(2):
                    for nt in range(RCH):
                        nc.tensor.matmul(
                            accs3[nt][:],
                            strip[:, u, P * nt : P * (nt + 1)],
                            t_sb[:, j + u, :],
                            start=False,
                            stop=False,
                        )
            # last two strips nt-major: accumulators complete staggered so
            # the relu/max/store epilogue pipelines behind the matmuls
            tail = []
            for j, g, c in LO_PAIRS[-2:]:
                strip = strips3.tile([P, 2, ROWS], cdt, name=f"rt_{j}",
                                     tag="strip3")
                nc.sync.dma_start(
                    strip[:].rearrange("p a b -> p (a b)"), s0p[j // 2]
                )
                tail.append((j, strip))
            for nt in range(RCH):
                for ti, (j, strip) in enumerate(tail):
                    for u in range(2):
                        nc.tensor.matmul(
                            accs3[nt][:],
                            strip[:, u, P * nt : P * (nt + 1)],
                            t_sb[:, j + u, :],
                            start=False,
                            stop=(ti == len(tail) - 1 and u == 1),
                        )
                lo = epi.tile([P, D], f32, name=f"elo_{nt}", tag="elo")
                nc.scalar.activation(
                    lo[:], accs3[nt][:], mybir.ActivationFunctionType.Relu,
                )
                osb = epi.tile([P, D], f32, name=f"osb_{nt}", tag="osb")
                nc.vector.tensor_tensor(
                    osb[:], lo[:], stash[nt][:], mybir.AluOpType.max
                )
                row0 = P * nt
                nc.sync.dma_start(out[row0 : row0 + P, :], osb[:])

    nc.compile()
    return nc


def _get_nc():
    if "nc" not in _CACHE:
        _CACHE["nc"] = _build_nc()
    return _CACHE["nc"]


def _shard_inputs(x, weights, alpha, bias, s0, s1, s2, s3):
    import ml_dtypes

    cnp = ml_dtypes.bfloat16

    def prep(a, scale=None):  # transpose (+ scale) + cast, C-contiguous
        t = a.T if scale is None else a.T * scale
        return np.ascontiguousarray(t).astype(cnp, copy=False)

    # softmax(alpha) folded into the low/high band matrices host-side
    af = np.asarray(alpha, dtype=np.float64)
    e = np.exp(af - af.max())
    a_sm = (e / e.sum()).astype(np.float32)

    seedb = np.ascontiguousarray(
        np.broadcast_to(np.asarray(bias, dtype=np.float32)[None, :], (P, D))
    )
    w_p = np.ascontiguousarray(weights).astype(cnp, copy=False)
    # xp[b, p, (j2 d)] = x[128*(2b+j2)+p, d]: 2KB contiguous per partition
    x_p = np.ascontiguousarray(
        x.astype(cnp, copy=False).reshape(NCH // 2, 2, P, D).transpose(0, 2, 1, 3)
        .reshape(NCH // 2, P, 2 * D)
    )

    def pack_A(t):  # t: [n, 1024] -> sweep A (i 0..511): [NB, P, SB*D]
        a = t[:, :D].reshape(NB, SB, P, D).transpose(0, 2, 1, 3)
        return np.ascontiguousarray(a.reshape(NB, P, SB * D))

    def pack_B(t):  # sweeps B1/B2 (i 512..767, 768..1023): [2, NB, P, SB*HD]
        a = t[:, D:].reshape(NB, SB, P, 2, HD).transpose(3, 0, 2, 1, 4)
        return np.ascontiguousarray(a.reshape(2, NB, P, SB * HD))

    def pack_pairs(t):  # t: [n, i] -> [n/256, P, 2*i]
        n, i = t.shape
        a = t.reshape(n // (2 * P), 2, P, i).transpose(0, 2, 1, 3)
        return np.ascontiguousarray(a.reshape(n // (2 * P), P, 2 * i))

    in_maps = []
    for c in range(N_CORES):
        r0, r1 = ROWS * c, ROWS * (c + 1)
        # S = concat(s1, s3) rows; core c owns rows [r0, r1)
        if r1 <= K:
            s_rows = s1[r0:r1]
        elif r0 >= K:
            s_rows = s3[r0 - K : r1 - K]
        else:  # straddles the boundary (not the case for these shapes)
            s_rows = np.concatenate([s1[r0:], s3[: r1 - K]], axis=0)
        st = prep(s_rows)
        in_maps.append(
            {
                "xp": x_p,
                "w": w_p,
                "seedb": seedb,
                "stpA": pack_A(st),
                "stpB": pack_B(st),
                "s0p": pack_pairs(prep(s0[r0:r1], a_sm[0])),
                "s2p": pack_pairs(prep(s2[r0:r1], a_sm[1])),
            }
        )
    return in_maps


def kernel(x, weights, alpha, bias, s0, s1, s2, s3, _trace=False):
    from concourse.bass_utils import run_bass_kernel_spmd

    nc = _get_nc()
    in_maps = _shard_inputs(
        np.asarray(x), np.asarray(weights), np.asarray(alpha), np.asarray(bias),
        np.asarray(s0), np.asarray(s1), np.asarray(s2), np.asarray(s3),
    )
    kwargs = {}
    if _trace:
        run_bass_kernel_spmd(nc, in_maps, core_ids=list(range(N_CORES)))
        kwargs = dict(trace=True, trace_cores=list(range(N_CORES)))
    r = run_bass_kernel_spmd(nc, in_maps, core_ids=list(range(N_CORES)), **kwargs)
    full = np.concatenate([res["out"] for res in r.results], axis=0)
    if _trace:
        return full, r
    return full
